# revision 1
# baseline (speedup 1.0000x reference)
"""Multi-head attention on 8 TRN2 NeuronCores.

Sharding: core c handles batch b = c // 4 and heads [4g, 4g+4) with g = c % 4.
Each core computes its 4 heads' contribution to out[b] = concat(heads) @ W_o;
the host sums the 4 per-batch partials and adds b_o.

Data path is fp16 (host-converted); all PE matmuls fp16 x fp16 -> fp32 PSUM
(1 cycle/row). Softmax stays fp32 where it matters (scores psum, normalizer).

Per-core dataflow:
  qT/kT/vT [1025, 2048] = [x[b].T ; ones-row]  (ones row folds the biases in)
  QT[e,s]  = (Wq_aug pair).T @ qT     -> PSUM -> SBUF fp16   [128, 2048] x2 pairs
  KT, VT   likewise (VT staged to SBUF fp16 per head + ones row -> [65, 2048])
  V[t,65]  = PE-transpose of VT per head (col 64 == 1.0)
  scoresT  = KT_h.T @ QT_h                     [t-tile 128, s]  (k = 64)
  msc      = scoresT * maskT_scaled            (DVE, psum(f32) x sbuf(f16))
  expT     = exp(msc)                          (ACT -> fp16)
  U        = attn@[V|1]: sum_t expT            [65, s] psum; row 64 = sum(exp)
  headsT   = U[0:64] * (1/U[64]) broadcast     (DVE -> fp16)
  out      += headsT(pair).T @ Wo_rows         [s-tile 128, 1024]
"""

import os
import numpy as np

B = 2
S = 2048
D = 1024
H = 16
DH = 64
DA = D + 1  # bias-augmented contraction dim
NCORES = 8
HPC = 4  # heads per core
SH = S // 2  # s-half processed per attention sweep
TT = S // 128  # 16 t-tiles

_cache = {}


def _build_program():
    import concourse.mybir as mybir
    import concourse.tile as tile
    from concourse import bacc
    from concourse.masks import make_identity

    f32 = mybir.dt.float32
    f16 = mybir.dt.float16

    nc = bacc.Bacc(None, target_bir_lowering=False, debug=False)
    qT = nc.declare_dram_parameter("qT", [DA, S], f16, isOutput=False)
    kT = nc.declare_dram_parameter("kT", [DA, S], f16, isOutput=False)
    vT = nc.declare_dram_parameter("vT", [DA, S], f16, isOutput=False)
    maskT = nc.declare_dram_parameter("maskT", [S, S], f16, isOutput=False)
    wq = nc.declare_dram_parameter("wq", [DA, 2 * 128], f16, isOutput=False)
    wk = nc.declare_dram_parameter("wk", [DA, 2 * 128], f16, isOutput=False)
    wv = nc.declare_dram_parameter("wv", [DA, 2 * 128], f16, isOutput=False)
    wo = nc.declare_dram_parameter("wo", [2 * 128, D], f16, isOutput=False)
    out = nc.declare_dram_parameter("out", [S, D], f32, isOutput=True)

    Exp = mybir.ActivationFunctionType.Exp

    with tile.TileContext(nc) as tc:
        with tc.tile_pool(name="persist", bufs=1) as pw:
            wq_sb = pw.tile([128, 9, 256], f16, tag="wq_sb")
            wk_sb = pw.tile([128, 9, 256], f16, tag="wk_sb")
            wv_sb = pw.tile([128, 9, 256], f16, tag="wv_sb")
            wo_sb = pw.tile([128, 2, D], f16, tag="wo_sb")
            ident = pw.tile([128, 128], f16, tag="ident")
            make_identity(nc, ident[:])
            QT_sb = pw.tile([128, 2, S], f16, tag="QT")
            KT_sb = pw.tile([128, 2, S], f16, tag="KT")
            V_sb = pw.tile([128, HPC, TT, 65], f16, tag="V")

            # ---- Phase B1: load weights, project Q/K/V ----
            with tc.tile_pool(name="maskpool", bufs=1) as mp:
              mask_tiles = [
                  mp.tile([128, TT, SH], f16, tag="mask", bufs=2, name=f"mask{s_}")
                  for s_ in range(2)
              ]

              def emit_mask_dmas(sh_):
                  for tt in range(TT):
                      nc.sync.dma_start(
                          mask_tiles[sh_][:, tt, :],
                          maskT[tt * 128 : (tt + 1) * 128, sh_ * SH : (sh_ + 1) * SH],
                      )

              with tc.tile_pool(name="vtpool", bufs=1) as vtp:
                vt_h = vtp.tile([65, HPC, S], f16, tag="vt_h")
                with (
                  tc.tile_pool(name="stage", bufs=3) as st,
                  tc.tile_pool(name="ps_proj", bufs=2, space="PSUM") as psp,
                ):
                  for ktile in range(9):
                      rows = 128 if ktile < 8 else 1
                      nc.sync.dma_start(
                          wq_sb[:rows, ktile, :],
                          wq[ktile * 128 : ktile * 128 + rows, :],
                      )
                      nc.sync.dma_start(
                          wk_sb[:rows, ktile, :],
                          wk[ktile * 128 : ktile * 128 + rows, :],
                      )
                      nc.sync.dma_start(
                          wv_sb[:rows, ktile, :],
                          wv[ktile * 128 : ktile * 128 + rows, :],
                      )
                  for ktile in range(2):
                      nc.sync.dma_start(
                          wo_sb[:, ktile, :], wo[ktile * 128 : (ktile + 1) * 128, :]
                      )

                  for x_dram, w_sb, dst, kind in (
                      (vT, wv_sb, None, "v"),
                      (qT, wq_sb, QT_sb, "q"),
                      (kT, wk_sb, KT_sb, "k"),
                  ):
                      accs = [
                          psp.tile([128, S], f32, tag="proj", name=f"acc_{kind}{pp}")
                          for pp in range(2)
                      ]
                      for ktile in range(9):
                          rows = 128 if ktile < 8 else 1
                          xst = st.tile([128, S], f16, tag="xst", bufs=4)
                          nc.sync.dma_start(
                              xst[:rows, :],
                              x_dram[ktile * 128 : ktile * 128 + rows, :],
                          )
                          for p in range(2):
                              for ch in range(4):
                                  cs = slice(ch * 512, (ch + 1) * 512)
                                  nc.tensor.matmul(
                                      accs[p][:, cs],
                                      w_sb[:rows, ktile, p * 128 : (p + 1) * 128],
                                      xst[:rows, cs],
                                      start=(ktile == 0),
                                      stop=(ktile == 8),
                                  )
                      if kind == "k":
                          emit_mask_dmas(0)
                          emit_mask_dmas(1)
                      for p in range(2):
                          if kind != "v":
                              nc.scalar.copy(dst[:, p, :], accs[p][:])
                          else:
                              for hh in range(2):
                                  h = p * 2 + hh
                                  nc.scalar.copy(
                                      vt_h[0:64, h, :],
                                      accs[p][hh * 64 : hh * 64 + 64, :],
                                  )
                                  nc.gpsimd.memset(vt_h[64:65, h, :], 1.0)

                # ---- Phase B2: per-head V transpose (with ones row) ----
                with tc.tile_pool(name="ps_vt", bufs=4, space="PSUM") as psv:
                  for h in range(HPC):
                      for tt in range(TT):
                          vps = psv.tile([128, 65], f16, tag="vps")
                          nc.tensor.transpose(
                              vps[:],
                              vt_h[0:65, h, tt * 128 : (tt + 1) * 128],
                              ident[0:65, 0:65],
                          )
                          nc.scalar.copy(V_sb[:, h, tt, :], vps[:])

              # ---- Phase C/D: attention + output projection per s-half ----
              with (
                  tc.tile_pool(name="attn", bufs=2) as at,
                  tc.tile_pool(name="ps_sc", bufs=2, space="PSUM") as pssc,
                  tc.tile_pool(name="ps_u", bufs=2, space="PSUM") as psu,
              ):
                  for sh in range(2):
                      s0 = sh * SH
                      mask_sb = mask_tiles[sh]
                      headsT = [
                          at.tile(
                              [128, SH], f16, tag="headsT", bufs=2, name=f"headsT{pp}"
                          )
                          for pp in range(2)
                      ]
                      for h in range(HPC):
                          p, hh = divmod(h, 2)
                          er = slice(hh * 64, hh * 64 + 64)
                          u_ps = psu.tile([65, SH], f32, tag="u")
                          for tt in range(TT):
                              sc = pssc.tile([128, SH], f32, tag="sc", bufs=2)
                              for ch in range(2):
                                  cs = slice(ch * 512, (ch + 1) * 512)
                                  nc.tensor.matmul(
                                      sc[:, cs],
                                      KT_sb[er, p, tt * 128 : (tt + 1) * 128],
                                      QT_sb[er, p, s0 + ch * 512 : s0 + (ch + 1) * 512],
                                      start=True,
                                      stop=True,
                                  )
                              msc = at.tile([128, SH], f32, tag="msc", bufs=4)
                              nc.vector.tensor_mul(msc[:], sc[:], mask_sb[:, tt, :])
                              expr = at.tile([128, SH], f16, tag="expr", bufs=4)
                              nc.scalar.activation(expr[:], msc[:], Exp)
                              for ch in range(2):
                                  cs = slice(ch * 512, (ch + 1) * 512)
                                  nc.tensor.matmul(
                                      u_ps[:, cs],
                                      V_sb[:, h, tt, :],
                                      expr[:, cs],
                                      start=(tt == 0),
                                      stop=(tt == TT - 1),
                                  )
                          nrec = at.tile([1, SH], f32, tag="nrec", bufs=2)
                          nc.vector.reciprocal(nrec[:], u_ps[64:65, :])
                          nb = at.tile([64, SH], f32, tag="nb", bufs=2)
                          nc.gpsimd.partition_broadcast(nb[:], nrec[:])
                          nc.vector.tensor_mul(headsT[p][er, :], u_ps[0:64, :], nb[:])

                      for st_i in range(SH // 128):
                          o_ps = psu.tile([128, D], f32, tag="u", bufs=2)
                          for p in range(2):
                              for ch in range(2):
                                  cs = slice(ch * 512, (ch + 1) * 512)
                                  nc.tensor.matmul(
                                      o_ps[:, cs],
                                      headsT[p][:, st_i * 128 : (st_i + 1) * 128],
                                      wo_sb[:, p, cs],
                                      start=(p == 0),
                                      stop=(p == 1),
                                  )
                          o_sb = at.tile([128, D], f32, tag="o_sb", bufs=2)
                          if st_i % 2 == 0:
                              nc.vector.tensor_copy(o_sb[:], o_ps[:])
                          else:
                              nc.scalar.copy(o_sb[:], o_ps[:])
                          nc.sync.dma_start(
                              out[s0 + st_i * 128 : s0 + (st_i + 1) * 128, :], o_sb[:]
                          )

    nc.finalize()
    return nc


def kernel(q, k, v, mask, W_q, b_q, W_k, b_k, W_v, b_v, W_o, b_o):
    from concourse.bass_utils import run_bass_kernel_spmd

    q = np.asarray(q, dtype=np.float32)
    k = np.asarray(k, dtype=np.float32)
    v = np.asarray(v, dtype=np.float32)
    mask = np.asarray(mask, dtype=np.float32)
    W_q = np.asarray(W_q, dtype=np.float32)
    b_q = np.asarray(b_q, dtype=np.float32)
    W_k = np.asarray(W_k, dtype=np.float32)
    b_k = np.asarray(b_k, dtype=np.float32)
    W_v = np.asarray(W_v, dtype=np.float32)
    b_v = np.asarray(b_v, dtype=np.float32)
    W_o = np.asarray(W_o, dtype=np.float32)
    b_o = np.asarray(b_o, dtype=np.float32)

    if "nc" not in _cache:
        _cache["nc"] = _build_program()
    nc = _cache["nc"]

    scale = 1.0 / np.sqrt(np.float32(DH))
    maskT = np.ascontiguousarray((mask.T * scale).astype(np.float16))

    def aug(x_b):  # [S, D] -> [D+1, S] fp16 with ones row
        return np.concatenate(
            [np.ascontiguousarray(x_b.T), np.ones((1, S), np.float32)], axis=0
        ).astype(np.float16)

    def w_aug(W, bvec, heads):  # -> [DA, 2*128] fp16 pair-stacked
        cols = []
        for p in range(2):
            h0, h1 = heads[2 * p], heads[2 * p + 1]
            wpair = np.concatenate([W[h0], W[h1]], axis=1)  # [D, 128]
            bpair = np.concatenate([bvec[h0], bvec[h1]])[None, :]  # [1, 128]
            cols.append(np.concatenate([wpair, bpair], axis=0))
        return np.ascontiguousarray(np.concatenate(cols, axis=1)).astype(np.float16)

    in_maps = []
    for c in range(NCORES):
        b, g = divmod(c, HPC)
        heads = list(range(HPC * g, HPC * g + HPC))
        in_maps.append(
            {
                "qT": aug(q[b]),
                "kT": aug(k[b]),
                "vT": aug(v[b]),
                "maskT": maskT,
                "wq": w_aug(W_q, b_q, heads),
                "wk": w_aug(W_k, b_k, heads),
                "wv": w_aug(W_v, b_v, heads),
                "wo": np.ascontiguousarray(
                    W_o[heads[0] * DH : (heads[-1] + 1) * DH]
                ).astype(np.float16),
            }
        )

    trace = bool(int(os.environ.get("KERNEL_TRACE", "0")))
    res = run_bass_kernel_spmd(nc, in_maps, list(range(NCORES)), trace=trace)
    _cache["last_results"] = res

    full = np.zeros((B, S, D), np.float32)
    for c in range(NCORES):
        full[c // HPC] += res.results[c]["out"]
    full += b_o[None, None, :]
    return full



# revision 38
# speedup vs baseline: 1.0597x; 1.0597x over previous
"""Multi-head attention on 8 TRN2 NeuronCores.

Sharding: core c handles batch b = c // 4 and heads [4g, 4g+4) with g = c % 4.
Each core computes its 4 heads' contribution to out[b] = concat(heads) @ W_o;
the host sums the 4 per-batch partials and adds b_o.

v7 dataflow (per core), f16 value path (fp8 fails the 2e-2 tolerance: noise
on any value tensor reaches the output unattenuated), engine-balanced
against the TRN2 cost model:
  - Projections f16; biases folded into the psum->SBUF staging copies
    (ACT Identity with per-partition bias on pair 0, DVE tensor_scalar on
    pair 1 to parallelize the prologue).
  - V projected directly in [t, e] orientation (x stage tile stationary,
    Wv moving): no transposes. V matmuls are emitted AFTER head 0's
    scores/mask/exp and its attn-V is deferred, so the PE starts attention
    as early as possible; V psum borrows the scores pool.
  - scores f16 (K=64) -> psum f32 -> DVE mask-mul (mask f16, pre-scaled by
    1/sqrt(dh)) -> msc f16.
  - exp on ACT in FD=4096 batches, bias=-2.0 (softmax-invariant shift),
    output f16 feeding attn@V against [V|ones]; ones column gives the
    denominators in row 64 of the psum.
  - normalize: DVE reciprocal + Pool partition_broadcast + ACT psum copy +
    Pool multiply (keeps the DVE free for the mask-muls).
  - out projection f16; output s-tiles interleave into the next s-half's
    attention; psum -> f16 SBUF copies alternate ACT/DVE in the tail;
    f16 DMA to DRAM; host sums partials in f32 and adds b_o.
"""

import os
import numpy as np

B = 2
S = 2048
D = 1024
H = 16
DH = 64
NCORES = 8
HPC = 4  # heads per core
SH = S // 2  # s-half processed per attention sweep
TT = S // 128  # 16 t-tiles
KT = 8  # k-tiles in the contraction (1024 = 8 * 128)
G = 4  # t-tiles per softmax group (exp FD = G*1024)

_cache = {}


def _build_program():
    import concourse.mybir as mybir
    import concourse.tile as tile
    from concourse import bacc

    f32 = mybir.dt.float32
    f16 = mybir.dt.float16
    Exp = mybir.ActivationFunctionType.Exp
    Ident = mybir.ActivationFunctionType.Identity
    Alu = mybir.AluOpType

    nc = bacc.Bacc(None, target_bir_lowering=False, debug=False)
    xq = nc.declare_dram_parameter("xq", [D, S], f16, isOutput=False)
    xk = nc.declare_dram_parameter("xk", [D, S], f16, isOutput=False)
    xv = nc.declare_dram_parameter("xv", [D, S], f16, isOutput=False)
    maskT = nc.declare_dram_parameter("maskT", [S, S], f16, isOutput=False)
    wq = nc.declare_dram_parameter("wq", [2, KT, 128, 128], f16, isOutput=False)
    wk = nc.declare_dram_parameter("wk", [2, KT, 128, 128], f16, isOutput=False)
    wv = nc.declare_dram_parameter("wv", [KT, 128, 256], f16, isOutput=False)
    bqkv = nc.declare_dram_parameter("bqkv", [128, 6], f32, isOutput=False)
    bvrow = nc.declare_dram_parameter("bvrow", [1, 256], f16, isOutput=False)
    wo = nc.declare_dram_parameter("wo", [2, 128, D], f16, isOutput=False)
    out = nc.declare_dram_parameter("out", [S, D], f16, isOutput=True)

    with tile.TileContext(nc) as tc:
        with (
            tc.tile_pool(name="persist", bufs=1) as pw,
            tc.tile_pool(name="stage", bufs=1) as st,
            tc.tile_pool(name="attn", bufs=2) as at,
        ):
            wq_sb = pw.tile([128, 2, KT, 128], f16, tag="wq_sb")
            wk_sb = pw.tile([128, 2, KT, 128], f16, tag="wk_sb")
            wv_sb = pw.tile([128, KT, 256], f16, tag="wv_sb")
            bq_sb = pw.tile([128, 6], f32, tag="bq_sb")
            wo_sb = pw.tile([128, 2, D], f16, tag="wo_sb")
            ones_r = pw.tile([1, 128], f16, tag="ones_r")
            bv_sb = pw.tile([1, 256], f16, tag="bv_sb")
            bias_m2 = pw.tile([128, 1], f32, tag="bias_m2")
            nc.gpsimd.memset(bias_m2[:], -2.0)
            nc.gpsimd.memset(ones_r[:], 1.0)
            QT = pw.tile([128, 2, S], f16, tag="QT", name="QT")
            KTs = pw.tile([128, 2, S], f16, tag="KTs", name="KTs")
            V8 = pw.tile([128, TT, HPC, 65], f16, tag="V8")
            mask_tiles = {}

            # ---- weight/bias loads; wq first so Q projection starts ASAP
            for p in range(2):
                nc.sync.dma_start(
                    wq_sb[:, p, :, :], wq[p].rearrange("kt p m -> p kt m")
                )
            nc.sync.dma_start(bq_sb[:, :], bqkv[:, :])
            nc.sync.dma_start(bv_sb[:, :], bvrow[:, :])

            psp_ctx = tc.tile_pool(name="ps_proj", bufs=2, space="PSUM")
            psp = psp_ctx.__enter__()
            stq_ctx = tc.tile_pool(name="qkstage", bufs=1)
            stq = stq_ctx.__enter__()

            def project(x_dram, w_sb, tag, accs, pool):
                xsts = [
                    pool.tile([128, 2, S], f16, tag=tag, bufs=2, name=f"{tag}{kp}")
                    for kp in range(KT // 2)
                ]
                for kp in range(KT // 2):
                    nc.sync.dma_start(
                        xsts[kp][:],
                        x_dram[kp * 256 : (kp + 1) * 256, :].rearrange(
                            "(i p) s -> p i s", p=128
                        ),
                    )
                for kp in range(KT // 2):
                    for i in range(2):
                        kt = 2 * kp + i
                        for p in range(2):
                            for ch in range(4):
                                cs = slice(ch * 512, (ch + 1) * 512)
                                nc.tensor.matmul(
                                    accs[p][:, cs],
                                    w_sb[:, p, kt, :],
                                    xsts[kp][:, i, cs],
                                    start=(kt == 0),
                                    stop=(kt == KT - 1),
                                )
                return xsts

            # ---- Phase B: project Q then K; stage with bias fold
            for x_dram, w_sb, wbi, kind in ((xq, wq_sb, 0, "q"), (xk, wk_sb, 1, "k")):
                accs = [
                    psp.tile([128, S], f32, tag="proj", name=f"acc_{kind}{pp}")
                    for pp in range(2)
                ]
                project(x_dram, w_sb, "xst" + kind, accs, stq)
                if kind == "q":  # prefetch K weights behind the xq stages
                    for p in range(2):
                        nc.sync.dma_start(
                            wk_sb[:, p, :, :],
                            wk[p].rearrange("kt p m -> p kt m"),
                        )
                dst = QT if kind == "q" else KTs
                nc.scalar.activation(
                    dst[:, 0, :], accs[0][:], Ident,
                    bias=bq_sb[:, 2 * wbi : 2 * wbi + 1], scale=1.0,
                )
                nc.vector.tensor_scalar(
                    dst[:, 1, :], accs[1][:], 1.0,
                    bq_sb[:, 2 * wbi + 1 : 2 * wbi + 2],
                    Alu.mult, Alu.add,
                )

            # mask half 0 via the ACT HWDGE queue, staggered behind x loads
            for g_ in range(TT // G):
                mt = at.tile(
                    [128, G, SH], f16, tag="mask", bufs=4, name=f"mask0_{g_}"
                )
                mask_tiles[(0, g_)] = mt
                with tc.tile_wait_until(0.014 + 0.003 * g_):
                    nc.scalar.dma_start(
                        mt[:],
                        maskT[
                            g_ * G * 128 : (g_ + 1) * G * 128, 0:SH
                        ].rearrange("(tt p) s -> p tt s", p=128),
                    )
            psp_ctx.__exit__(None, None, None)
            stq_ctx.__exit__(None, None, None)

            # V weight/x loads early (transfers overlap h0 attention);
            # V matmuls themselves are emitted inside the attention region.
            nc.sync.dma_start(wv_sb[:, :, :], wv[:].rearrange("kt k e -> k kt e"))
            xstv = [
                st.tile([128, 2, S], f16, tag="xstv", bufs=KT // 2, name=f"xstv{kp}")
                for kp in range(KT // 2)
            ]
            for kp in range(KT // 2):
                nc.sync.dma_start(
                    xstv[kp][:],
                    xv[kp * 256 : (kp + 1) * 256, :].rearrange(
                        "(i p) s -> p i s", p=128
                    ),
                )
            for g_ in range(TT // G):
                mt = at.tile(
                    [128, G, SH], f16, tag="mask", bufs=4, name=f"mask1_{g_}"
                )
                mask_tiles[(1, g_)] = mt
                nc.sync.dma_start(
                    mt[:],
                    maskT[
                        g_ * G * 128 : (g_ + 1) * G * 128, SH:S
                    ].rearrange("(tt p) s -> p tt s", p=128),
                )
            for p in range(2):
                nc.sync.dma_start(wo_sb[:, p, :], wo[p])

            # ---- Phase C/D: attention + output projection per s-half ----
            with (
                tc.tile_pool(name="ps_sc", bufs=2, space="PSUM") as pssc,
                tc.tile_pool(name="ps_u", bufs=2, space="PSUM") as psu,
            ):

                def emit_v_chunk(c):
                    # V in [t, e] orientation; psum borrowed from the sc pool
                    for tt in range(4 * c, 4 * c + 4):
                        vps = pssc.tile([128, 256], f32, tag="sc", bufs=2)
                        for kp in range(KT // 2):
                            for i in range(2):
                                kt = 2 * kp + i
                                nc.tensor.matmul(
                                    vps[:],
                                    xstv[kp][:, i, tt * 128 : (tt + 1) * 128],
                                    wv_sb[:, kt, :],
                                    start=(kt == 0),
                                    stop=False,
                                )
                        nc.tensor.matmul(
                            vps[:], ones_r[:, :], bv_sb[:, :], start=False, stop=True
                        )
                        nc.scalar.copy(
                            V8[:, tt, :, 0:64],
                            vps[:, :].rearrange("p (h e) -> p h e", h=HPC),
                        )
                    if c == 3:
                        for h in range(HPC):
                            nc.gpsimd.memset(V8[:, :, h, 64:65], 1.0)

                def emit_av(u_ps, h, g, expr):
                    for i in range(G):
                        tt = g * G + i
                        for ch in range(2):
                            cs = slice(ch * 512, (ch + 1) * 512)
                            nc.tensor.matmul(
                                u_ps[:, cs],
                                V8[:, tt, h, 0:65],
                                expr[:, i, cs],
                                start=(tt == 0),
                                stop=(tt == TT - 1),
                            )

                def phase_d_unit(sh, headsT, st_i, tail):
                    s0 = sh * SH
                    o_ps = psu.tile([128, D], f32, tag="u", bufs=2)
                    for p in range(2):
                        for ch in range(2):
                            cs = slice(ch * 512, (ch + 1) * 512)
                            nc.tensor.matmul(
                                o_ps[:, cs],
                                headsT[p][:, st_i * 128 : (st_i + 1) * 128],
                                wo_sb[:, p, cs],
                                start=(p == 0),
                                stop=(p == 1),
                            )
                    o_sb = at.tile([128, D], f16, tag="o_sb", bufs=2)
                    if tail and st_i % 2 == 1:
                        nc.vector.tensor_copy(o_sb[:], o_ps[:])
                    else:
                        nc.scalar.copy(o_sb[:], o_ps[:])
                    nc.sync.dma_start(
                        out[s0 + st_i * 128 : s0 + (st_i + 1) * 128, :], o_sb[:]
                    )

                pending = []
                for sh in range(2):
                    s0 = sh * SH
                    headsT = [
                        at.tile(
                            [128, SH], f16, tag="headsT", bufs=4, name=f"hT{sh}{pp}"
                        )
                        for pp in range(2)
                    ]
                    for h in range(HPC):
                        p, hh = divmod(h, 2)
                        er = slice(hh * 64, hh * 64 + 64)
                        defer = sh == 0 and h == 0
                        u_ps = psu.tile([65, SH], f32, tag="u")
                        av_jobs = []
                        for g in range(TT // G):
                            if defer:
                                emit_v_chunk(g)
                            msc = at.tile([128, G, SH], f16, tag="msc", bufs=2)
                            expr = at.tile([128, G, SH], f16, tag="expr", bufs=4)
                            for i in range(G):
                                tt = g * G + i
                                sc = pssc.tile([128, SH], f32, tag="sc", bufs=2)
                                for ch in range(2):
                                    cs = slice(ch * 512, (ch + 1) * 512)
                                    nc.tensor.matmul(
                                        sc[:, cs],
                                        KTs[er, p, tt * 128 : (tt + 1) * 128],
                                        QT[
                                            er, p,
                                            s0 + ch * 512 : s0 + (ch + 1) * 512,
                                        ],
                                        start=True,
                                        stop=True,
                                    )
                                nc.vector.tensor_mul(
                                    msc[:, i, :], sc[:], mask_tiles[(sh, g)][:, i, :]
                                )
                            nc.scalar.activation(expr[:], msc[:], Exp, bias=bias_m2[:])
                            if defer:
                                av_jobs.append((g, expr))
                            else:
                                emit_av(u_ps, h, g, expr)
                        for g2, expr2 in av_jobs:
                            emit_av(u_ps, h, g2, expr2)
                        nrec = at.tile([1, SH], f16, tag="nrec", bufs=1)
                        with nc.allow_low_precision(
                            "softmax denominators are O(100); f16 reciprocal "
                            "keeps 0.05% relative error"
                        ):
                            nc.vector.reciprocal(nrec[:], u_ps[64:65, :])
                        nb = at.tile([64, SH], f16, tag="nb", bufs=1)
                        nc.gpsimd.partition_broadcast(nb[:], nrec[:])
                        ucp = at.tile([64, SH], f16, tag="ucp", bufs=1)
                        nc.scalar.copy(ucp[:], u_ps[0:64, :])
                        nc.gpsimd.tensor_tensor(
                            headsT[p][er, :], ucp[:], nb[:], Alu.mult
                        )
                        for _ in range(2):
                            if pending:
                                pending.pop(0)()

                    pending = [
                        (lambda sh=sh, headsT=headsT, st_i=st_i: phase_d_unit(
                            sh, headsT, st_i, sh == 1
                        ))
                        for st_i in range(SH // 128)
                    ]
                for f in pending:
                    f()

    nc.finalize()
    return nc


def kernel(q, k, v, mask, W_q, b_q, W_k, b_k, W_v, b_v, W_o, b_o):
    from concourse.bass_utils import run_bass_kernel_spmd

    q = np.asarray(q, dtype=np.float32)
    k = np.asarray(k, dtype=np.float32)
    v = np.asarray(v, dtype=np.float32)
    mask = np.asarray(mask, dtype=np.float32)
    W_q = np.asarray(W_q, dtype=np.float32)
    b_q = np.asarray(b_q, dtype=np.float32)
    W_k = np.asarray(W_k, dtype=np.float32)
    b_k = np.asarray(b_k, dtype=np.float32)
    W_v = np.asarray(W_v, dtype=np.float32)
    b_v = np.asarray(b_v, dtype=np.float32)
    W_o = np.asarray(W_o, dtype=np.float32)
    b_o = np.asarray(b_o, dtype=np.float32)

    if "nc" not in _cache:
        _cache["nc"] = _build_program()
    nc = _cache["nc"]

    scale = 1.0 / np.sqrt(np.float32(DH))
    maskTh = np.ascontiguousarray((mask.T * scale).astype(np.float16))

    def xT16(x_b):  # [S, D] -> [D, S] f16
        return np.ascontiguousarray(x_b.T).astype(np.float16)

    def w16(W, heads):  # [H, D, DH] -> [2, KT, 128, 128] f16
        cols = []
        for pp in range(2):
            h0, h1 = heads[2 * pp], heads[2 * pp + 1]
            wpair = np.concatenate([W[h0], W[h1]], axis=1)  # [D, 128]
            cols.append(wpair.reshape(KT, 128, 128))
        return np.ascontiguousarray(np.stack(cols, axis=0)).astype(np.float16)

    def wv16(W, heads):  # [H, D, DH] -> [KT, 128, 256] f16
        wcat = np.concatenate([W[h] for h in heads], axis=1)  # [D, 256]
        return np.ascontiguousarray(wcat.reshape(KT, 128, 256)).astype(np.float16)

    def bcat(bvec, heads):  # [H, DH] -> [128, 2] f32 (pair-concat per column)
        return np.stack(
            [
                np.concatenate([bvec[heads[2 * pp]], bvec[heads[2 * pp + 1]]])
                for pp in range(2)
            ],
            axis=1,
        ).astype(np.float32)

    in_maps = []
    for c in range(NCORES):
        b, g = divmod(c, HPC)
        heads = list(range(HPC * g, HPC * g + HPC))
        in_maps.append(
            {
                "xq": xT16(q[b]),
                "xk": xT16(k[b]),
                "xv": xT16(v[b]),
                "maskT": maskTh,
                "wq": w16(W_q, heads),
                "wk": w16(W_k, heads),
                "wv": wv16(W_v, heads),
                "bvrow": np.ascontiguousarray(
                    np.concatenate([b_v[h] for h in heads])[None, :]
                ).astype(np.float16),
                "bqkv": np.ascontiguousarray(
                    np.concatenate(
                        [bcat(b_q, heads), bcat(b_k, heads), bcat(b_v, heads)],
                        axis=1,
                    )
                ),
                "wo": np.ascontiguousarray(
                    W_o[heads[0] * DH : (heads[-1] + 1) * DH].reshape(2, 128, D)
                ).astype(np.float16),
            }
        )

    trace = bool(int(os.environ.get("KERNEL_TRACE", "0")))
    res = run_bass_kernel_spmd(nc, in_maps, list(range(NCORES)), trace=trace)
    _cache["last_results"] = res

    full = np.zeros((B, S, D), np.float32)
    for c in range(NCORES):
        full[c // HPC] += np.asarray(res.results[c]["out"], dtype=np.float32)
    full += b_o[None, None, :]
    return full
